# revision 36
# baseline (speedup 1.0000x reference)
"""Trainium2 Bass kernel for nn_Attention_44564580663760.

Single-head "attention" (B=8, S=2048, D=1024, fp32) with the reference's
quirk reproduced: scores = q @ v^T (k projection unused), causal mask,
softmax, ctx @ v, output projection.

Sharding: data-parallel - one batch element per NeuronCore (8 cores).

Fused algebraic form (exact, softmax row-constants cancel):
    M = Wq^T Wv          scores ~ x M x^T * scale + g[j],  g = (x Wv^T bq)*scale
    N = Wv^T Wo^T        values-out vo = x N,  h = exp(g)
    out = (E @ (h .* vo)) / (E @ h) + (bv Wo^T + bo),  E = exp(x M x^T * scale)
This removes the separate q/v/out projections: 4 device GEMMs total
(~557k PE cycles/core vs ~690k for the direct form). h (a length-S
vector) and the fused weights M, N, bo'' are precomputed on host.

Per-core dataflow (contraction is always the partition dim; no transposes,
no DRAM scratch round trips):
    P: pT[d, s]  = M-stationary @ xT        (8 psum accum passes)
    V: vo[s, d]  = h .* (xT-stationary @ N) (natural layout, no transpose)
    S: eT[j, i]  = exp(scale * xT-stationary @ pT), causal-ragged tiles
    O: out[i, d] = eT-stationary @ vo, l = eT-stationary @ h (shared LDW);
       out = out/l + bo''
"""

import os
import sys

sys.path.insert(0, "/opt/trn_rl_repo")

import contextlib

import numpy as np

PREWARM = int(os.environ.get("K_PREWARM", "4"))
DUALRING = os.environ.get("K_DUALRING", "1") == "1"
NODEDUP = os.environ.get("K_NODEDUP", "0") == "1"

import concourse.bacc as bacc
import concourse.bass as bass
import concourse.mybir as mybir
import concourse.tile as tile
from concourse.bass_utils import run_bass_kernel_spmd

FP32 = mybir.dt.float32
BF16 = mybir.dt.bfloat16

B, S, D = 8, 2048, 1024
PT = 128
NTS = S // PT  # 16 s-tiles
NTD = D // PT  # 8 d-tiles
SCALE = 1.0 / np.sqrt(np.float32(D))
ACT = mybir.ActivationFunctionType


def _chunks(lo, hi, step=512):
    """Bank-aligned column chunks covering [lo, hi)."""
    out = []
    c = lo
    while c < hi:
        e = min(hi, (c // step + 1) * step)
        out.append((c, e))
        c = e
    return out


def build_nc(causal: bool, reps: int = 0) -> bass.Bass:
    nc = bacc.Bacc("TRN2", target_bir_lowering=False, debug=False)
    dram = {
        "xT": nc.declare_dram_parameter("xT", [D, S], BF16, isOutput=False),
        "Mt": nc.declare_dram_parameter("Mt", [D, D], BF16, isOutput=False),
        "Nt": nc.declare_dram_parameter("Nt", [D, D], BF16, isOutput=False),
        "hf": nc.declare_dram_parameter("hf", [PT, NTS], FP32, isOutput=False),
        "hb": nc.declare_dram_parameter("hb", [PT, NTS], BF16, isOutput=False),
        "bob": nc.declare_dram_parameter("bob", [PT, D], FP32, isOutput=False),
        "utri": nc.declare_dram_parameter("utri", [PT, PT], BF16, isOutput=False),
        "out": nc.declare_dram_parameter("out", [S, D], FP32, isOutput=True),
    }

    with tile.TileContext(nc) as tc:
        loop_ctx = tc.For_i(0, reps, 1) if reps else contextlib.nullcontext()
        with loop_ctx:
            _body(nc, tc, causal, dram)
    if not NODEDUP:
        _dedup_ldweights(nc)
    nc.finalize()
    return nc


def _dedup_ldweights(nc):
    """Drop InstLdweights whose stationary operand matches the previous PE
    weight load (no intervening PE weight change) - the paired matmuls then
    reuse the already-loaded weights. Deps of a dropped LDW move to the next
    kept instruction so semaphore generation still orders correctly."""
    removed = {}
    for bb in nc.main_func.blocks:
        insts = bb.instructions
        keep = []
        last_sig = None
        pending = []
        for ins in insts:
            drop = False
            if isinstance(ins, mybir.InstLdweights):
                sig = (
                    str(ins.ins[0]),
                    bool(ins.is_transpose),
                    str(ins.perf_mode),
                    str(ins.tile_position),
                )
                if sig == last_sig:
                    drop = True
                else:
                    last_sig = sig
            elif (
                getattr(ins, "engine", None) == mybir.EngineType.PE
                and isinstance(ins, mybir.InstMatmult)
                and ins.is_transpose
            ):
                last_sig = None
            if drop:
                pending.append(ins)
                continue
            for p in pending:
                ins.merge_dependencies_from(p)
                removed[p.name] = ins.name
            pending = []
            keep.append(ins)
        assert not pending
        if len(keep) != len(insts):
            insts[:] = keep
    if removed:
        for bb in nc.main_func.blocks:
            for ins in bb.instructions:
                ins.remap_dependency_names(removed)
        if hasattr(nc, "inst_map"):
            for name in removed:
                nc.inst_map.pop(name, None)


def _body(nc, tc, causal, dram):
    with (
        tc.tile_pool(name="xTp", bufs=1) as xTp,
        tc.tile_pool(name="pTp", bufs=1) as pTp,
        tc.tile_pool(name="vop", bufs=1) as vop,
        tc.tile_pool(name="eTp", bufs=1) as eTp,
        tc.tile_pool(name="hp", bufs=1) as hp,
        tc.tile_pool(name="constp", bufs=1) as constp,
        tc.tile_pool(name="outp", bufs=2) as outp,
    ):
        # ---- loads, split across the two HWDGE rings (SP=sync, ACT=scalar):
        # sync ring: xT first (phase P/V/S critical), then small consts
        # scalar ring: M (phase P), then N (phase V)
        ring2 = nc.scalar if DUALRING else nc.sync
        xT_t, M_t = [], []
        for k in range(NTD):
            xt = xTp.tile([PT, S], BF16, tag=f"xT{k}", name=f"xT{k}")
            # chunk the first tiles so phase P can start as soon as the
            # leading columns land (a whole-tile DMA gates on the last byte)
            nch = 4 if k == 0 else (2 if k == 1 else 1)
            for c0, c1 in _chunks(0, S, S // nch):
                nc.sync.dma_start(
                    xt[:, c0:c1], dram["xT"][k * PT : (k + 1) * PT, c0:c1]
                )
            xT_t.append(xt)
        utri_t = constp.tile([PT, PT], BF16, tag="utri", name="utri")
        nc.sync.dma_start(utri_t[:], dram["utri"][:, :])
        bob_t = constp.tile([PT, D], FP32, tag="bob", name="bob")
        nc.sync.dma_start(bob_t[:], dram["bob"][:, :])
        hf_t = hp.tile([PT, NTS], FP32, tag="hf", name="hf")
        nc.sync.dma_start(hf_t[:], dram["hf"][:, :])
        hb_t = hp.tile([PT, NTS], BF16, tag="hb", name="hb")
        nc.sync.dma_start(hb_t[:], dram["hb"][:, :])

        with tc.tile_pool(name="Mp", bufs=1) as Mp:
            for k in range(NTD):
                mt = Mp.tile([PT, D], BF16, tag=f"M{k}", name=f"M{k}")
                ring2.dma_start(mt[:], dram["Mt"][k * PT : (k + 1) * PT, :])
                M_t.append(mt)
            N_t = []
            with tc.tile_pool(name="Np", bufs=1) as Np:
                for k in range(NTD):
                    nt = Np.tile([PT, D], BF16, tag=f"N{k}", name=f"N{k}")
                    ring2.dma_start(nt[:], dram["Nt"][k * PT : (k + 1) * PT, :])
                    N_t.append(nt)

                # ---- Phase P: pT[e] = sum_k Mt[k][:, e-slice].T @ xT[k] ----
                pT_t = [
                    pTp.tile([PT, S], BF16, tag=f"pT{e}", name=f"pT{e}")
                    for e in range(NTD)
                ]
                with tc.tile_pool(name="psP", bufs=2, space="PSUM") as psPp:
                    # PE pre-warm: junk matmuls while the first loads land
                    # (HAM un-throttles after ~3.4us of PE activity; these
                    # also fill the initial DMA-wait bubble).
                    if PREWARM:
                        junk = constp.tile([PT, 512], BF16, tag="junk", name="junk")
                        nc.gpsimd.memset(junk[:], 0.0)
                        psw = psPp.tile([PT, S], FP32, tag="ps", name="ps")
                        for _ in range(PREWARM):
                            nc.tensor.matmul(
                                psw[:, 0:512],
                                junk[:, 0:PT],
                                junk[:],
                                start=True,
                                stop=True,
                            )
                    for e in range(NTD):
                        ps = psPp.tile([PT, S], FP32, tag="ps", name="ps")
                        for k in range(NTD):
                            for c0, c1 in _chunks(0, S):
                                nc.tensor.matmul(
                                    ps[:, c0:c1],
                                    M_t[k][:, e * PT : (e + 1) * PT],
                                    xT_t[k][:, c0:c1],
                                    start=(k == 0),
                                    stop=(k == NTD - 1),
                                )
                        nc.scalar.activation(
                            pT_t[e][:], ps[:], ACT.Identity, scale=1.0
                        )

                # ---- Phase V: vo[st] = h[st] * (xT-stat @ N) ----
                vo_t = [
                    vop.tile([PT, D], BF16, tag=f"vo{st}", name=f"vo{st}")
                    for st in range(NTS)
                ]
                with tc.tile_pool(name="psV", bufs=3, space="PSUM") as psVp:
                    for st in range(NTS):
                        psv = psVp.tile([PT, D], FP32, tag="v", name="v")
                        for k in range(NTD):
                            lhs = xT_t[k][:, st * PT : (st + 1) * PT]
                            for c0, c1 in _chunks(0, D):
                                nc.tensor.matmul(
                                    psv[:, c0:c1],
                                    lhs,
                                    N_t[k][:, c0:c1],
                                    start=(k == 0),
                                    stop=(k == NTD - 1),
                                )
                        nc.vector.tensor_scalar_mul(
                            vo_t[st][:], psv[:], hf_t[:, st : st + 1]
                        )

        # ---- Phase S: eT[ki] = exp(scale * xT-stat @ pT), causal ragged ----
        eT_t = []
        with tc.tile_pool(name="psS", bufs=2, space="PSUM") as psSp:
            for ki in range(NTS):
                lo = ki * PT if causal else 0
                ps = psSp.tile([PT, S], FP32, tag="s", name="s")
                for k in range(NTD):
                    lhs = xT_t[k][:, ki * PT : (ki + 1) * PT]
                    for c0, c1 in _chunks(lo, S):
                        nc.tensor.matmul(
                            ps[:, c0:c1],
                            lhs,
                            pT_t[k][:, c0:c1],
                            start=(k == 0),
                            stop=(k == NTD - 1),
                        )
                et = eTp.tile([PT, S - lo], BF16, tag=f"e{ki}", name=f"e{ki}")
                nc.scalar.activation(et[:], ps[:, lo:S], ACT.Exp, scale=float(SCALE))
                if causal:
                    nc.vector.tensor_mul(et[:, 0:PT], et[:, 0:PT], utri_t[:])
                eT_t.append(et)

        # ---- Phase O: out[it] = eT-stat @ vo (+ l column, shared LDW) ----
        with (
            tc.tile_pool(name="psO", bufs=3, space="PSUM") as psOp,
            tc.tile_pool(name="psL", bufs=2, space="PSUM") as psLp,
        ):
            # interleave long/short accumulation chains so any two adjacent
            # its have enough PE work to hide the per-it DVE+DMA drain, and
            # the kernel doesn't end on a run of short chains.
            it_order = []
            for a, b in zip(reversed(range(NTS // 2, NTS)), range(NTS // 2)):
                it_order += [a, b]
            for it in (it_order if causal else range(NTS)):
                kmax = (it + 1) if causal else NTS
                pso = psOp.tile([PT, D], FP32, tag="o", name="o")
                psl = psLp.tile([PT, 1], FP32, tag="l", name="l")
                for ki in range(kmax):
                    lo = ki * PT if causal else 0
                    lhs = eT_t[ki][:, it * PT - lo : (it + 1) * PT - lo]
                    for c0, c1 in _chunks(0, D):
                        nc.tensor.matmul(
                            pso[:, c0:c1],
                            lhs,
                            vo_t[ki][:, c0:c1],
                            start=(ki == 0),
                            stop=(ki == kmax - 1),
                        )
                    nc.tensor.matmul(
                        psl[:],
                        lhs,
                        hb_t[:, ki : ki + 1],
                        start=(ki == 0),
                        stop=(ki == kmax - 1),
                    )
                rl = hp.tile([PT, 1], FP32, tag="rl", name="rl")
                nc.vector.reciprocal(rl[:], psl[:])
                os_ = outp.tile([PT, D], FP32, tag="os", name="os")
                # split the normalize+bias+store into column halves across
                # both DVE->DMA chains/rings to shorten the serial tail
                for half, (c0, c1) in enumerate(_chunks(0, D)):
                    nc.vector.tensor_scalar_mul(os_[:, c0:c1], pso[:, c0:c1], rl[:])
                    nc.vector.tensor_add(
                        os_[:, c0:c1], os_[:, c0:c1], bob_t[:, c0:c1]
                    )
                    eng = nc.scalar if (DUALRING and half % 2 == 1) else nc.sync
                    eng.dma_start(
                        dram["out"][it * PT : (it + 1) * PT, c0:c1],
                        os_[:, c0:c1],
                    )


_TRIL = None


def _detect_causal(mask: np.ndarray) -> bool:
    global _TRIL
    m0 = np.asarray(mask[0])
    if bool(m0[0, 1]):
        if not m0.all() or not np.asarray(mask).all():
            raise NotImplementedError("unsupported mask pattern")
        return False
    if _TRIL is None:
        _TRIL = np.tril(np.ones((S, S), dtype=bool))
    for b in range(mask.shape[0]):
        if not np.array_equal(np.asarray(mask[b]), _TRIL):
            raise NotImplementedError("unsupported mask pattern")
    return True


def _host_prep(x, Wq, bq, Wv, bv, Wo, bo):
    import ml_dtypes

    f32 = np.float32
    WqT = np.asarray(Wq, f32).T  # [din, dout] as applied to x
    WvT = np.asarray(Wv, f32).T
    WoT = np.asarray(Wo, f32).T
    M = WqT @ WvT.T  # [din, din']
    N = WvT @ WoT  # [din, dout]
    c = WvT @ np.asarray(bq, f32)  # [din]
    bo2 = np.asarray(bv, f32) @ WoT + np.asarray(bo, f32)  # [dout]
    base = {
        "Mt": M.astype(ml_dtypes.bfloat16),
        "Nt": N.astype(ml_dtypes.bfloat16),
        "bob": np.tile(bo2.reshape(1, D), (PT, 1)).astype(f32),
        "utri": np.triu(np.ones((PT, PT), dtype=f32)).astype(ml_dtypes.bfloat16),
    }
    in_maps = []
    for b in range(B):
        xb = np.asarray(x[b], f32)
        h = np.exp((xb @ c) * f32(SCALE)).astype(f32)  # [S], j-indexed
        hcol = np.ascontiguousarray(h.reshape(NTS, PT).T)  # [PT, NTS]
        in_maps.append(
            {
                "xT": np.ascontiguousarray(xb.T).astype(ml_dtypes.bfloat16),
                "hf": hcol,
                "hb": hcol.astype(ml_dtypes.bfloat16),
                **base,
            }
        )
    return in_maps


def kernel(x, mask, Wq, bq, Wk, bk, Wv, bv, Wo, bo):
    causal = _detect_causal(np.asarray(mask))
    nc = build_nc(causal)
    in_maps = _host_prep(x, Wq, bq, Wv, bv, Wo, bo)
    res = run_bass_kernel_spmd(nc, in_maps, list(range(B)))
    out = np.stack([np.asarray(res.results[i]["out"]) for i in range(B)])
    return out.astype(np.float32)


if __name__ == "__main__":
    rng = np.random.default_rng(0)
    x = rng.standard_normal((B, S, D), dtype=np.float32)
    mask = np.broadcast_to(np.tril(np.ones((S, S), dtype=bool)), (B, S, S))
    mk = lambda *s: (rng.standard_normal(s, dtype=np.float32) * 0.02)
    out = kernel(
        x, mask, mk(D, D), mk(D), mk(D, D), mk(D), mk(D, D), mk(D), mk(D, D), mk(D)
    )
    print(out.shape, out.dtype)
